# revision 3
# baseline (speedup 1.0000x reference)
"""GCN conv block (gather -> normalized scatter-add -> matmul -> bias ->
LeakyReLU -> BatchNorm) on 8 Trainium2 NeuronCores.

Strategy: nodes are partitioned across the 8 cores (graph parallel). Edges are
bucketed by destination tile (128 nodes) on host into a padded CSR-like layout;
x and the [128,128] weight are replicated. On device, each core:
  - computes dinv = 1/sqrt(1 + in_degree) for all nodes (from rowptr input),
  - for each of its destination tiles: indirect-DMA gathers x[src] rows and
    dinv[src] scalars, builds a weighted one-hot matrix R[e,d] =
    (dst_local[e]==d) * dinv[src[e]] on the fly, and accumulates
    X_g^T @ R in PSUM -> the aggregated (features x nodes) tile,
  - scales columns by dinv[dst] (broadcast tile), applies W, bias and
    LeakyReLU (as relu(z) - 0.01*relu(-z), with fused BN sum accumulators),
  - AllReduces the per-feature BN statistics across cores, and applies the
    final affine before storing the output tile (features x nodes).
Host only shards/pads index structures and reassembles the output.
"""
import sys

for _p in ("/opt/trn_rl_repo",):
    if _p not in sys.path:
        sys.path.insert(0, _p)

import numpy as np

from concourse import bass, bacc, mybir
import concourse.tile as tile
from concourse.bass_utils import run_bass_kernel_spmd

P = 128
D = 128
N_CORES = 8
LEAKY = 0.01
BN_EPS = 1e-5

F32 = mybir.dt.float32
I32 = mybir.dt.int32

LAST_RESULTS = None  # BassKernelResults of the most recent run (for harnesses)


def _build_program(n_nodes, tpc, K):
    """Build the SPMD program. tpc = dst tiles per core, K = edge chunks/tile.

    Table sizes: dinv table covers n_slots = ceil(n_nodes/P)*P ... we use
    128*ceil(n_nodes*... simplified: n_tab = number of node slots covered by
    the dinv table, a multiple of P*?? -- we pass n_tab explicitly derived
    from n_nodes (see _prep).
    """
    n_tab = ((n_nodes + P - 1) // P) * P  # dinv table rows (>= n_nodes)
    # free-dim width for the [128, n_tab//128] setup layout
    ncol = n_tab // P
    n_own = tpc * P  # nodes per core

    nc = bacc.Bacc("TRN2", target_bir_lowering=False, debug=False)

    x_d = nc.dram_tensor("x", [n_nodes, D], F32, kind="ExternalInput")
    rowptr_d = nc.dram_tensor("rowptr", [n_tab + 1, 1], F32, kind="ExternalInput")
    rowptr_own_d = nc.dram_tensor("rowptr_own", [n_own + 1, 1], F32, kind="ExternalInput")
    src_idx_d = nc.dram_tensor("src_idx", [tpc, P, K], I32, kind="ExternalInput")
    dst_loc_d = nc.dram_tensor("dst_loc", [tpc, P, K], F32, kind="ExternalInput")
    nphant_d = nc.dram_tensor("nphant", [P, 1], F32, kind="ExternalInput")
    w_d = nc.dram_tensor("W", [D, D], F32, kind="ExternalInput")
    b_d = nc.dram_tensor("b", [D, 1], F32, kind="ExternalInput")
    gamma_d = nc.dram_tensor("gamma", [D, 1], F32, kind="ExternalInput")
    beta_d = nc.dram_tensor("beta", [D, 1], F32, kind="ExternalInput")

    out_d = nc.dram_tensor("out", [tpc, D, P], F32, kind="ExternalOutput")

    dinv_full_d = nc.dram_tensor("dinv_full", [n_tab, 1], F32)
    dinv_own_d = nc.dram_tensor("dinv_own", [1, n_own], F32)
    cc_in_d = nc.dram_tensor("cc_in", [P, 2], F32)
    cc_out_d = nc.dram_tensor("cc_out", [P, 2], F32)

    inv_n = 1.0 / float(n_nodes)

    with tile.TileContext(nc) as tc:
        with (
            tc.tile_pool(name="persist", bufs=1) as pp,
            tc.tile_pool(name="setup", bufs=1) as sp,
            tc.tile_pool(name="idxp", bufs=3) as idxp,
            tc.tile_pool(name="xgp", bufs=3) as xgp,
            tc.tile_pool(name="rp", bufs=4) as rpool,
            tc.tile_pool(name="actp", bufs=3) as actp,
            tc.tile_pool(name="finp", bufs=3) as finp,
            tc.tile_pool(name="psA", bufs=2, space="PSUM") as psA,
            tc.tile_pool(name="psC", bufs=2, space="PSUM") as psC,
        ):
            # ---------- setup: dinv tables ----------
            rp0 = sp.tile([P, ncol], dtype=F32)
            rp1 = sp.tile([P, ncol], dtype=F32)
            rpf = rowptr_d[:].flatten()
            nc.sync.dma_start(out=rp0[:], in_=rpf[0:n_tab].rearrange("(p c) -> p c", p=P))
            nc.sync.dma_start(out=rp1[:], in_=rpf[1 : n_tab + 1].rearrange("(p c) -> p c", p=P))
            degm = sp.tile([P, ncol], dtype=F32)
            nc.vector.tensor_tensor(out=degm[:], in0=rp1[:], in1=rp0[:], op=mybir.AluOpType.subtract)
            sq = sp.tile([P, ncol], dtype=F32)
            nc.scalar.activation(sq[:], degm[:], mybir.ActivationFunctionType.Sqrt, bias=1.0)
            dinv_sb = sp.tile([P, ncol], dtype=F32)
            nc.vector.reciprocal(dinv_sb[:], sq[:])
            nc.sync.dma_start(
                out=dinv_full_d[:].flatten()[0:n_tab].rearrange("(p c) -> p c", p=P),
                in_=dinv_sb[:],
            )

            ro0 = sp.tile([P, tpc], dtype=F32)
            ro1 = sp.tile([P, tpc], dtype=F32)
            rof = rowptr_own_d[:].flatten()
            nc.sync.dma_start(out=ro0[:], in_=rof[0:n_own].rearrange("(p c) -> p c", p=P))
            nc.sync.dma_start(out=ro1[:], in_=rof[1 : n_own + 1].rearrange("(p c) -> p c", p=P))
            degmo = sp.tile([P, tpc], dtype=F32)
            nc.vector.tensor_tensor(out=degmo[:], in0=ro1[:], in1=ro0[:], op=mybir.AluOpType.subtract)
            sqo = sp.tile([P, tpc], dtype=F32)
            nc.scalar.activation(sqo[:], degmo[:], mybir.ActivationFunctionType.Sqrt, bias=1.0)
            dinvo_sb = sp.tile([P, tpc], dtype=F32)
            nc.vector.reciprocal(dinvo_sb[:], sqo[:])
            nc.sync.dma_start(
                out=dinv_own_d[:].flatten().rearrange("(p c) -> p c", p=P),
                in_=dinvo_sb[:],
            )
            # broadcast dinv_own across all 128 partitions: [128, n_own]
            dinvb = pp.tile([P, n_own], dtype=F32)
            nc.sync.dma_start(out=dinvb[:], in_=dinv_own_d[0:1, :].to_broadcast([P, n_own]))

            # ---------- setup: constants ----------
            iota_i = sp.tile([P, P], dtype=I32)
            nc.gpsimd.iota(iota_i[:], pattern=[[1, P]], base=0, channel_multiplier=0)
            iota_f = pp.tile([P, P], dtype=F32)
            nc.vector.tensor_copy(iota_f[:], iota_i[:])

            w_sb = pp.tile([D, D], dtype=F32)
            nc.sync.dma_start(out=w_sb[:], in_=w_d[:])
            b_sb = pp.tile([D, 1], dtype=F32)
            nc.sync.dma_start(out=b_sb[:], in_=b_d[:])
            nb_sb = pp.tile([D, 1], dtype=F32)
            nc.vector.tensor_scalar(nb_sb[:], b_sb[:], -1.0, None, mybir.AluOpType.mult)
            gamma_sb = pp.tile([D, 1], dtype=F32)
            nc.sync.dma_start(out=gamma_sb[:], in_=gamma_d[:])
            beta_sb = pp.tile([D, 1], dtype=F32)
            nc.sync.dma_start(out=beta_sb[:], in_=beta_d[:])
            nph_sb = pp.tile([P, 1], dtype=F32)
            nc.sync.dma_start(out=nph_sb[:], in_=nphant_d[:])

            out_big = pp.tile([P, n_own], dtype=F32)
            spos = pp.tile([P, tpc], dtype=F32)
            sneg = pp.tile([P, tpc], dtype=F32)
            qpos = pp.tile([P, tpc], dtype=F32)
            qneg = pp.tile([P, tpc], dtype=F32)

            # ---------- main loop over destination tiles ----------
            for t in range(tpc):
                blk = slice(t * P, (t + 1) * P)
                idx_sb = idxp.tile([P, K], dtype=I32)
                nc.sync.dma_start(out=idx_sb[:], in_=src_idx_d[t])
                dstl_sb = idxp.tile([P, K], dtype=F32)
                nc.sync.dma_start(out=dstl_sb[:], in_=dst_loc_d[t])
                g1 = idxp.tile([P, K], dtype=F32)
                xg = xgp.tile([P, K * D], dtype=F32)
                for k in range(K):
                    nc.gpsimd.indirect_dma_start(
                        out=g1[:, k : k + 1],
                        out_offset=None,
                        in_=dinv_full_d[:],
                        in_offset=bass.IndirectOffsetOnAxis(ap=idx_sb[:, k : k + 1], axis=0),
                    )
                    nc.gpsimd.indirect_dma_start(
                        out=xg[:, k * D : (k + 1) * D],
                        out_offset=None,
                        in_=x_d[:],
                        in_offset=bass.IndirectOffsetOnAxis(ap=idx_sb[:, k : k + 1], axis=0),
                    )
                pa = psA.tile([P, P], dtype=F32, space="PSUM")
                for k in range(K):
                    r_sb = rpool.tile([P, P], dtype=F32)
                    nc.vector.tensor_scalar(
                        r_sb[:],
                        iota_f[:],
                        dstl_sb[:, k : k + 1],
                        g1[:, k : k + 1],
                        mybir.AluOpType.is_equal,
                        mybir.AluOpType.mult,
                    )
                    nc.tensor.matmul(
                        out=pa[:],
                        lhsT=xg[:, k * D : (k + 1) * D],
                        rhs=r_sb[:],
                        start=(k == 0),
                        stop=(k == K - 1),
                    )
                at_sb = actp.tile([P, P], dtype=F32)
                nc.vector.tensor_tensor(
                    out=at_sb[:], in0=pa[:], in1=dinvb[:, blk], op=mybir.AluOpType.mult
                )
                pc = psC.tile([P, P], dtype=F32, space="PSUM")
                nc.tensor.matmul(out=pc[:], lhsT=w_sb[:], rhs=at_sb[:], start=True, stop=True)
                pos = actp.tile([P, P], dtype=F32)
                neg = actp.tile([P, P], dtype=F32)
                scr = actp.tile([P, P], dtype=F32)
                nc.scalar.activation(
                    pos[:], pc[:], mybir.ActivationFunctionType.Relu,
                    bias=b_sb[:], scale=1.0, accum_out=spos[:, t : t + 1],
                )
                nc.scalar.activation(
                    neg[:], pc[:], mybir.ActivationFunctionType.Relu,
                    bias=nb_sb[:], scale=-1.0, accum_out=sneg[:, t : t + 1],
                )
                nc.scalar.activation(
                    scr[:], pos[:], mybir.ActivationFunctionType.Square,
                    accum_out=qpos[:, t : t + 1],
                )
                scr2 = actp.tile([P, P], dtype=F32)
                nc.scalar.activation(
                    scr2[:], neg[:], mybir.ActivationFunctionType.Square,
                    accum_out=qneg[:, t : t + 1],
                )
                nc.vector.tensor_scalar(
                    neg[:], neg[:], LEAKY, None, mybir.AluOpType.mult
                )
                nc.vector.tensor_tensor(
                    out=out_big[:, blk], in0=pos[:], in1=neg[:], op=mybir.AluOpType.subtract
                )

            # ---------- BN stats + AllReduce ----------
            rsp = sp.tile([P, 1], dtype=F32)
            rsn = sp.tile([P, 1], dtype=F32)
            rqp = sp.tile([P, 1], dtype=F32)
            rqn = sp.tile([P, 1], dtype=F32)
            nc.vector.tensor_reduce(rsp[:], spos[:], mybir.AxisListType.X, mybir.AluOpType.add)
            nc.vector.tensor_reduce(rsn[:], sneg[:], mybir.AxisListType.X, mybir.AluOpType.add)
            nc.vector.tensor_reduce(rqp[:], qpos[:], mybir.AxisListType.X, mybir.AluOpType.add)
            nc.vector.tensor_reduce(rqn[:], qneg[:], mybir.AxisListType.X, mybir.AluOpType.add)
            # sum(lrelu) = rsp - LEAKY*rsn ; sum(lrelu^2) = rqp + LEAKY^2*rqn
            s_lr = sp.tile([P, 1], dtype=F32)
            nc.vector.tensor_scalar(
                s_lr[:], rsn[:], -LEAKY, None, mybir.AluOpType.mult
            )
            nc.vector.tensor_tensor(out=s_lr[:], in0=s_lr[:], in1=rsp[:], op=mybir.AluOpType.add)
            q_lr = sp.tile([P, 1], dtype=F32)
            nc.vector.tensor_scalar(
                q_lr[:], rqn[:], LEAKY * LEAKY, None, mybir.AluOpType.mult
            )
            nc.vector.tensor_tensor(out=q_lr[:], in0=q_lr[:], in1=rqp[:], op=mybir.AluOpType.add)
            # subtract phantom-column contribution: nphant * lrelu(b) (per feature)
            pb = sp.tile([P, 1], dtype=F32)
            nb2 = sp.tile([P, 1], dtype=F32)
            nc.scalar.activation(pb[:], b_sb[:], mybir.ActivationFunctionType.Relu)
            nc.scalar.activation(nb2[:], b_sb[:], mybir.ActivationFunctionType.Relu, scale=-1.0)
            lb = sp.tile([P, 1], dtype=F32)
            nc.vector.tensor_scalar(lb[:], nb2[:], -LEAKY, None, mybir.AluOpType.mult)
            nc.vector.tensor_tensor(out=lb[:], in0=lb[:], in1=pb[:], op=mybir.AluOpType.add)
            lb2 = sp.tile([P, 1], dtype=F32)
            nc.scalar.activation(lb2[:], lb[:], mybir.ActivationFunctionType.Square)
            corr = sp.tile([P, 1], dtype=F32)
            nc.vector.tensor_tensor(out=corr[:], in0=nph_sb[:], in1=lb[:], op=mybir.AluOpType.mult)
            nc.vector.tensor_tensor(out=s_lr[:], in0=s_lr[:], in1=corr[:], op=mybir.AluOpType.subtract)
            nc.vector.tensor_tensor(out=corr[:], in0=nph_sb[:], in1=lb2[:], op=mybir.AluOpType.mult)
            nc.vector.tensor_tensor(out=q_lr[:], in0=q_lr[:], in1=corr[:], op=mybir.AluOpType.subtract)

            cc_sb = sp.tile([P, 2], dtype=F32)
            nc.vector.tensor_copy(cc_sb[:, 0:1], s_lr[:])
            nc.vector.tensor_copy(cc_sb[:, 1:2], q_lr[:])
            nc.sync.dma_start(out=cc_in_d[:], in_=cc_sb[:])
            nc.gpsimd.collective_compute(
                "AllReduce",
                mybir.AluOpType.add,
                replica_groups=[list(range(N_CORES))],
                ins=[cc_in_d[:]],
                outs=[cc_out_d[:]],
            )
            st = sp.tile([P, 2], dtype=F32)
            nc.sync.dma_start(out=st[:], in_=cc_out_d[:])
            mean = sp.tile([P, 1], dtype=F32)
            nc.vector.tensor_scalar(mean[:], st[:, 0:1], inv_n, None, mybir.AluOpType.mult)
            msq = sp.tile([P, 1], dtype=F32)
            nc.vector.tensor_scalar(msq[:], st[:, 1:2], inv_n, None, mybir.AluOpType.mult)
            m2 = sp.tile([P, 1], dtype=F32)
            nc.scalar.activation(m2[:], mean[:], mybir.ActivationFunctionType.Square)
            var = sp.tile([P, 1], dtype=F32)
            nc.vector.tensor_tensor(out=var[:], in0=msq[:], in1=m2[:], op=mybir.AluOpType.subtract)
            nc.vector.tensor_scalar(var[:], var[:], BN_EPS, None, mybir.AluOpType.add)
            sd = sp.tile([P, 1], dtype=F32)
            nc.scalar.activation(sd[:], var[:], mybir.ActivationFunctionType.Sqrt)
            rstd = sp.tile([P, 1], dtype=F32)
            nc.vector.reciprocal(rstd[:], sd[:])
            sfac = sp.tile([P, 1], dtype=F32)
            nc.vector.tensor_tensor(out=sfac[:], in0=gamma_sb[:], in1=rstd[:], op=mybir.AluOpType.mult)
            tsh = sp.tile([P, 1], dtype=F32)
            nc.vector.tensor_tensor(out=tsh[:], in0=mean[:], in1=sfac[:], op=mybir.AluOpType.mult)
            nc.vector.tensor_tensor(out=tsh[:], in0=beta_sb[:], in1=tsh[:], op=mybir.AluOpType.subtract)

            # ---------- final affine + store ----------
            for t in range(tpc):
                blk = slice(t * P, (t + 1) * P)
                fin = finp.tile([P, P], dtype=F32)
                nc.scalar.activation(
                    fin[:], out_big[:, blk], mybir.ActivationFunctionType.Identity,
                    bias=tsh[:], scale=sfac[:],
                )
                nc.sync.dma_start(out=out_d[t], in_=fin[:])

    nc.compile()
    return nc


def _prep(x, edge_index, n_nodes, tpc):
    """Host-side sharding: bucket edges by destination tile, pad, transpose."""
    n_tiles = N_CORES * tpc
    n_pad = n_tiles * P
    n_tab = ((n_nodes + P - 1) // P) * P

    src = np.ascontiguousarray(edge_index[0]).astype(np.int64)
    dst = np.ascontiguousarray(edge_index[1]).astype(np.int64)
    order = np.argsort(dst, kind="stable")
    src_s = src[order].astype(np.int32)
    dst_s = dst[order].astype(np.int32)

    counts = np.bincount(dst_s, minlength=n_pad)
    rowptr = np.zeros(n_pad + 1, np.int64)
    np.cumsum(counts, out=rowptr[1:])

    tb = rowptr[np.arange(n_tiles + 1) * P]
    nvalid = np.clip(n_nodes - np.arange(n_tiles) * P, 0, P).astype(np.int64)
    seg = np.diff(tb) + nvalid
    K = int(np.ceil(seg.max() / P))

    src_pad = np.zeros((n_tiles, K * P), np.int32)
    dstl_pad = np.full((n_tiles, K * P), -1.0, np.float32)
    for t in range(n_tiles):
        e0, e1 = tb[t], tb[t + 1]
        nv = nvalid[t]
        s = src_s[e0:e1]
        dl = (dst_s[e0:e1] - t * P).astype(np.float32)
        if nv:
            s = np.concatenate([s, np.arange(t * P, t * P + nv, dtype=np.int32)])
            dl = np.concatenate([dl, np.arange(nv, dtype=np.float32)])
        src_pad[t, : len(s)] = s
        dstl_pad[t, : len(dl)] = dl
    src_tpk = src_pad.reshape(n_tiles, K, P).transpose(0, 2, 1).copy()
    dstl_tpk = dstl_pad.reshape(n_tiles, K, P).transpose(0, 2, 1).copy()

    rowptr_f = rowptr[: n_tab + 1].astype(np.float32)

    in_maps = []
    for c in range(N_CORES):
        lo = c * tpc * P
        hi = lo + tpc * P
        nph = float(max(0, min(hi, n_pad) - max(lo, n_nodes)))
        in_maps.append(
            {
                "x": x,
                "rowptr": rowptr_f.reshape(-1, 1),
                "rowptr_own": rowptr[lo : hi + 1].astype(np.float32).reshape(-1, 1),
                "src_idx": src_tpk[c * tpc : (c + 1) * tpc],
                "dst_loc": dstl_tpk[c * tpc : (c + 1) * tpc],
                "nphant": np.full((P, 1), nph, np.float32),
            }
        )
    return in_maps, K


def _run(x, edge_index, W, b, gamma, beta, n_nodes, tpc, trace=False, tmpdir=None):
    global LAST_RESULTS
    in_maps, K = _prep(x, edge_index, n_nodes, tpc)
    for m in in_maps:
        m["W"] = np.ascontiguousarray(W, np.float32)
        m["b"] = np.ascontiguousarray(b, np.float32).reshape(D, 1)
        m["gamma"] = np.ascontiguousarray(gamma, np.float32).reshape(D, 1)
        m["beta"] = np.ascontiguousarray(beta, np.float32).reshape(D, 1)
    nc = _build_program(n_nodes, tpc, K)
    res = run_bass_kernel_spmd(
        nc, in_maps, list(range(N_CORES)), trace=trace, tmpdir=tmpdir
    )
    LAST_RESULTS = res
    blocks = [
        res.results[c]["out"].transpose(0, 2, 1).reshape(tpc * P, D)
        for c in range(N_CORES)
    ]
    return np.concatenate(blocks, axis=0)[:n_nodes]


def kernel(x, edge_index, W, b, gamma, beta):
    x = np.ascontiguousarray(x, np.float32)
    n_nodes = x.shape[0]
    tpc = (n_nodes + N_CORES * P - 1) // (N_CORES * P)
    out = _run(x, edge_index, W, b, gamma, beta, n_nodes, tpc)
    return out.astype(np.float32)
